# revision 35
# baseline (speedup 1.0000x reference)
"""ACT loss-head kernel for 8 TRN2 NeuronCores (data-parallel over batch).

Layout per core (16 of 128 batch rows):
  - tokens flattened row-major: flat = b_local*4096 + s, partition p holds
    tokens [512p, 512p+512)  => row b = p//8, s = (p%8)*512 + tau.
  - logits shard in SBUF as X [128, 512, 64] f32 (131KB/partition).

Per-core pipeline:
  - VectorE: per-token max over V; per-v (1..31) one-hot masks + predicated
    label-logit gather; per-v (2..27) set-membership counts (labels/preds/facts);
    per-partition accumulations of mask counts, correct counts, (lse-xl)*mask.
  - TensorE+ScalarE: PE-transpose X chunks to PSUM, exp on ScalarE (bf16 out),
    ones-matmul accumulates per-token sum(exp) into a [64,1024] PSUM block,
    Ln + PE-transpose back to token layout.
  - TensorE: row-selector matmul contracts per-partition stats to per-row stats.
  - Final per-row metric/loss assembly on [16,*] tiles; host sums 8x16 rows.
"""

import sys
import os
import contextlib

sys.path.insert(0, "/opt/trn_rl_repo")

import numpy as np

import concourse.bass as bass
import concourse.bacc as bacc
import concourse.tile as tile
from concourse import mybir, bass_isa, library_config
from concourse.masks import make_identity
from concourse.bass_utils import run_bass_kernel_spmd

F32 = mybir.dt.float32
I32 = mybir.dt.int32
BF16 = mybir.dt.bfloat16
U8 = mybir.dt.uint8
I16 = mybir.dt.int16
ALU = mybir.AluOpType
ACT = mybir.ActivationFunctionType
AX = mybir.AxisListType

P = 128          # partitions
TPP = 512        # tokens per partition
V = 64           # vocab
NROW = 16        # batch rows per core
SPR = 8          # partitions per row
NEG_BIG = -1e30
POS_BIG = 1e30
CHUNK_T = 16     # tokens per transpose/exp chunk (free dim 1024)
NCHUNK = TPP // CHUNK_T          # 32
DMA_T = 64       # tokens per DMA slice
NDMA = TPP // DMA_T              # 8
NSTAT = 108
# stat columns
C_LCNT = 0    # 26: label set counts (sliced)
C_FCNT = 26   # 26: facts set counts
C_PCNT = 52   # 26: pred set counts (sliced), half A
C_PCNT2 = 78  # 26: half B
C_MASK = 104
C_CORR = 105
C_LMRW = 106
NOUT = 8


def build_nc():
    nc = bacc.Bacc(None, target_bir_lowering=False)
    x_d = nc.declare_dram_parameter("logits", [P, TPP * V], F32, isOutput=False)
    lab_d = nc.declare_dram_parameter("labels", [P, TPP], I32, isOutput=False)
    sm_d = nc.declare_dram_parameter("smalls", [NROW, 4], F32, isOutput=False)
    out_d = nc.declare_dram_parameter("out", [NROW, NOUT], F32, isOutput=True)

    with tile.TileContext(nc) as tc:
        with contextlib.ExitStack() as ctx:
            pool = ctx.enter_context(tc.tile_pool(name="main", bufs=1))
            dpool = ctx.enter_context(tc.tile_pool(name="dbl", bufs=2))
            psum = ctx.enter_context(tc.tile_pool(name="ps", bufs=1, space="PSUM"))
            psd = ctx.enter_context(tc.tile_pool(name="psd", bufs=2, space="PSUM"))
            jpool = ctx.enter_context(tc.tile_pool(name="junkp", bufs=4))
            ohlp = ctx.enter_context(tc.tile_pool(name="ohlp", bufs=16))

            # ---------- input tiles ----------
            lab_i = pool.tile([P, TPP], I32)
            nc.sync.dma_start(out=lab_i, in_=lab_d.ap())
            smalls = pool.tile([NROW, 4], F32)
            nc.sync.dma_start(out=smalls, in_=sm_d.ap())
            X = pool.tile([P, TPP, V], F32)
            x3 = x_d.ap().rearrange("p (t v) -> p t v", t=TPP)
            for i in range(NDMA):
                sl = slice(DMA_T * i, DMA_T * (i + 1))
                nc.sync.dma_start(out=X[:, sl, :], in_=x3[:, sl, :])

            # ---------- constants ----------
            ident = pool.tile([P, P], F32)
            make_identity(nc, ident)
            rowsel = pool.tile([P, NROW], F32)
            nc.gpsimd.memset(rowsel, 1.0)
            nc.gpsimd.affine_select(out=rowsel, in_=rowsel, compare_op=ALU.is_ge,
                                    fill=0.0, base=0, pattern=[[-SPR, NROW]],
                                    channel_multiplier=1)
            nc.gpsimd.affine_select(out=rowsel, in_=rowsel, compare_op=ALU.is_ge,
                                    fill=0.0, base=SPR - 1, pattern=[[SPR, NROW]],
                                    channel_multiplier=-1)
            Z2 = pool.tile([P, 130], BF16)
            nc.vector.memset(Z2, 0.0)
            nc.vector.memset(Z2[0:64, 64:65], 1.0)
            nc.vector.memset(Z2[64:128, 65:66], 1.0)
            pos_i = pool.tile([P, TPP], I32)
            nc.gpsimd.iota(pos_i, pattern=[[1, TPP]], base=0, channel_multiplier=TPP)
            nc.vector.tensor_scalar(out=pos_i, in0=pos_i, scalar1=4095, scalar2=None,
                                    op0=ALU.bitwise_and)
            pos_f = pool.tile([P, TPP], F32)
            nc.vector.tensor_copy(pos_f, pos_i)
            neg_pos = pool.tile([P, TPP], F32)
            nc.vector.tensor_scalar(out=neg_pos, in0=pos_f, scalar1=-1.0, scalar2=None,
                                    op0=ALU.mult)
            bigpos = pool.tile([P, 1], F32)
            nc.vector.memset(bigpos, POS_BIG)

            # ---------- label prep ----------
            lab_f = pool.tile([P, TPP], BF16)
            nc.vector.tensor_copy(lab_f, lab_i)
            mask01 = pool.tile([P, TPP], BF16)
            nc.vector.tensor_scalar(out=mask01, in0=lab_f, scalar1=0.5, scalar2=None,
                                    op0=ALU.is_gt)

            # ---------- per-token max over V (with interleaved XL preds) ----------
            HT = TPP // 2
            XL = pool.tile([P, TPP], F32)
            nc.gpsimd.memset(XL, NEG_BIG)
            early_masks = {}
            for v in range(1, 13):
                eqv = ohlp.tile([P, TPP], I16, tag="ohl", name="eqv")
                nc.vector.tensor_scalar(out=eqv, in0=lab_f, scalar1=float(v),
                                        scalar2=None, op0=ALU.is_equal)
                early_masks[v] = eqv
            QT = HT // 2
            M = pool.tile([P, TPP], F32)
            for i in range(NDMA):
                sl = slice(DMA_T * i, DMA_T * (i + 1))
                nc.vector.reduce_max(M[:, sl], X[:, sl, :], axis=AX.X)
                if i in (2, 3):
                    for v in range(6 * (i - 2) + 1, 6 * (i - 1) + 1):
                        nc.vector.copy_predicated(XL[:, 0:QT],
                                                  early_masks[v][:, 0:QT],
                                                  X[:, 0:QT, v])
                elif i >= 4:
                    for v in range(3 * (i - 4) + 1, 3 * (i - 3) + 1):
                        nc.vector.copy_predicated(XL[:, QT:HT],
                                                  early_masks[v][:, QT:HT],
                                                  X[:, QT:HT, v])

            # ---------- first-occurrence masks (slice / facts) ----------
            def junk_t():
                jk = jpool.tile([P, TPP], BF16, tag="junkout", name="jk")
                return jk

            def junk_f():
                jf = jpool.tile([P, TPP], F32, tag="junkf", name="jf")
                return jf

            def first_occurrence_neg(eq_mask):
                # returns [P,1] = max over tokens of (-pos where eq else -BIG)
                scr = jpool.tile([P, TPP], F32, tag="scratch", name="scr")
                nc.gpsimd.memset(scr, NEG_BIG)
                nc.vector.copy_predicated(scr, eq_mask, neg_pos)
                out_pp = pool.tile([P, 1], F32, tag="fo")
                nc.vector.reduce_max(out_pp, scr, axis=AX.X)
                return out_pp

            def row_allreduce_max(val_pp):
                # [P,1] -> per-row max broadcast back to all partitions of the row
                g = pool.tile([NROW, SPR], F32, tag="rowred")
                nc.vector.dma_start(out=g, in_=val_pp)
                r16 = pool.tile([NROW, 1], F32, tag="rowred2")
                nc.vector.reduce_max(r16, g, axis=AX.X)
                bk = pool.tile([P, 1], F32, tag="rowred3")
                nc.vector.dma_start(out=bk, in_=r16.broadcast_to((NROW, SPR)))
                return bk

            e28 = pool.tile([P, TPP], I16, tag="eqv8")
            nc.vector.tensor_scalar(out=e28, in0=lab_f, scalar1=28.0, scalar2=None,
                                    op0=ALU.is_equal)
            n28 = row_allreduce_max(first_occurrence_neg(e28))
            fstart = pool.tile([P, 1], F32)
            nc.vector.tensor_scalar(out=fstart, in0=n28, scalar1=-1.0, scalar2=1.0,
                                    op0=ALU.mult, op1=ALU.add)  # min28+1 or BIG

            e30 = pool.tile([P, TPP], I16, tag="eqv8")
            nc.vector.tensor_scalar(out=e30, in0=lab_f, scalar1=30.0, scalar2=None,
                                    op0=ALU.is_equal)
            n30 = row_allreduce_max(first_occurrence_neg(e30))
            f30 = pool.tile([P, 1], F32)
            nc.vector.tensor_scalar(out=f30, in0=n30, scalar1=-0.9e30, scalar2=None,
                                    op0=ALU.is_gt)             # found flag
            t30a = pool.tile([P, 1], F32)
            nc.vector.tensor_scalar(out=t30a, in0=n30, scalar1=-1.0, scalar2=None,
                                    op0=ALU.mult)
            t30b = pool.tile([P, 1], F32)
            nc.vector.tensor_tensor(out=t30b, in0=t30a, in1=f30, op=ALU.mult)
            t30 = pool.tile([P, 1], F32)
            nc.vector.scalar_tensor_tensor(out=t30, in0=f30, scalar=1.0, in1=t30b,
                                           op0=ALU.subtract, op1=ALU.add)  # -1 if none
            slice01_b = pool.tile([P, TPP], BF16)
            nc.vector.tensor_scalar(out=slice01_b, in0=pos_f, scalar1=t30, scalar2=None,
                                    op0=ALU.is_gt)

            edel = pool.tile([P, TPP], BF16, tag="eqv2")
            nc.vector.tensor_scalar(out=edel, in0=lab_f, scalar1=29.0, scalar2=None,
                                    op0=ALU.is_equal)
            e31 = pool.tile([P, TPP], BF16, tag="eqv")
            nc.vector.tensor_scalar(out=e31, in0=lab_f, scalar1=31.0, scalar2=None,
                                    op0=ALU.is_equal)
            nc.vector.tensor_tensor(out=edel, in0=edel, in1=e31, op=ALU.add)
            gtf = pool.tile([P, TPP], BF16, tag="eqv")
            nc.vector.tensor_scalar(out=gtf, in0=pos_f, scalar1=fstart, scalar2=None,
                                    op0=ALU.is_gt)
            nc.vector.tensor_tensor(out=edel, in0=edel, in1=gtf, op=ALU.mult)
            edel_u8 = pool.tile([P, TPP], I16, tag="eqv8")
            nc.vector.tensor_copy(edel_u8, edel)
            ndel = row_allreduce_max(first_occurrence_neg(edel_u8))
            fdel = pool.tile([P, 1], F32)
            nc.vector.tensor_scalar(out=fdel, in0=ndel, scalar1=-0.9e30, scalar2=None,
                                    op0=ALU.is_gt)
            fend1 = pool.tile([P, 1], F32)
            nc.vector.tensor_scalar(out=fend1, in0=ndel, scalar1=-1.0, scalar2=None,
                                    op0=ALU.mult)
            fend = pool.tile([P, 1], F32)
            nc.vector.tensor_tensor(out=fend, in0=fend1, in1=fdel, op=ALU.mult)
            facts01_b = pool.tile([P, TPP], BF16)
            nc.vector.tensor_scalar(out=facts01_b, in0=pos_f, scalar1=fstart,
                                    scalar2=None, op0=ALU.is_ge)
            fl = pool.tile([P, TPP], BF16, tag="eqv")
            nc.vector.tensor_scalar(out=fl, in0=pos_f, scalar1=fend, scalar2=None,
                                    op0=ALU.is_lt)
            nc.vector.tensor_tensor(out=facts01_b, in0=facts01_b, in1=fl, op=ALU.mult)

            lab_s = pool.tile([P, TPP], BF16)
            nc.vector.tensor_tensor(out=lab_s, in0=lab_f, in1=slice01_b, op=ALU.mult)
            lab_fc = pool.tile([P, TPP], BF16)
            nc.vector.tensor_tensor(out=lab_fc, in0=lab_f, in1=facts01_b, op=ALU.mult)
            Ms = pool.tile([P, TPP], F32)
            notsl = pool.tile([P, TPP], I16, tag="eqv8")
            nc.vector.tensor_scalar(out=notsl, in0=slice01_b, scalar1=0.5, scalar2=None,
                                    op0=ALU.is_lt)
            for h in range(2):
                hs = slice(HT * h, HT * (h + 1))
                nc.vector.tensor_copy(Ms[:, hs], M[:, hs])
                nc.vector.copy_predicated(Ms[:, hs], notsl[:, hs],
                                          bigpos.broadcast_to((P, HT)))

            # ---------- stats buffer ----------
            stats = pool.tile([P, NSTAT], F32)
            nc.gpsimd.memset(stats, 0.0)

            # ---------- per-v loops ----------
            for v in range(1, 32):
                if v in early_masks:
                    eqv = early_masks[v]
                    halves = (1,)          # h0 already done in the max loop
                else:
                    eqv = ohlp.tile([P, TPP], I16, tag="ohl", name="eqv")
                    nc.vector.tensor_scalar(out=eqv, in0=lab_f, scalar1=float(v),
                                            scalar2=None, op0=ALU.is_equal)
                    halves = (0, 1)
                for h in halves:
                    hs = slice(HT * h, HT * (h + 1))
                    nc.vector.copy_predicated(XL[:, hs], eqv[:, hs], X[:, hs, v])
            for v in range(2, 28):
                nc.vector.tensor_scalar(out=junk_t(), in0=lab_s, scalar1=float(v),
                                        scalar2=None, op0=ALU.is_equal, op1=ALU.add,
                                        accum_out=stats[:, C_LCNT + v - 2: C_LCNT + v - 1])
            for v in range(2, 28):
                nc.vector.tensor_scalar(out=junk_t(), in0=lab_fc, scalar1=float(v),
                                        scalar2=None, op0=ALU.is_equal, op1=ALU.add,
                                        accum_out=stats[:, C_FCNT + v - 2: C_FCNT + v - 1])
            for h, base in ((0, C_PCNT), (1, C_PCNT2)):
                hs = slice(HT * h, HT * (h + 1))
                for v in range(2, 28):
                    nc.vector.scalar_tensor_tensor(
                        out=junk_t()[:, 0:HT], in0=X[:, hs, v], scalar=0.0,
                        in1=Ms[:, hs], op0=ALU.bypass, op1=ALU.is_ge,
                        accum_out=stats[:, base + v - 2: base + v - 1])

            nc.vector.tensor_scalar(out=junk_t(), in0=mask01, scalar1=0.0, scalar2=None,
                                    op0=ALU.add, op1=ALU.add,
                                    accum_out=stats[:, C_MASK:C_MASK + 1])
            nc.vector.scalar_tensor_tensor(
                out=junk_t(), in0=XL, scalar=0.0, in1=M,
                op0=ALU.bypass, op1=ALU.is_equal,
                accum_out=stats[:, C_CORR:C_CORR + 1])

            # ---------- transpose/exp/sumexp pipeline ----------
            SET = psum.tile([64, P * SPR], F32)  # [64, 1024]
            for c in range(NCHUNK):
                ta = psd.tile([P, P * SPR], F32, tag="ta")
                for b in range(SPR):
                    t0 = CHUNK_T * c + 2 * b
                    nc.tensor.transpose(
                        ta[:, P * b:P * (b + 1)],
                        X[:, t0:t0 + 2, :].rearrange("p a b -> p (a b)"),
                        ident)
                et = dpool.tile([P, P * SPR], BF16, tag="et")
                nc.scalar.activation(et, ta, ACT.Exp)
                for h in range(2):
                    hs = slice(512 * h, 512 * (h + 1))
                    nc.tensor.matmul(SET[:, hs], Z2[:, 64 - 2 * c: 128 - 2 * c],
                                     et[:, hs],
                                     start=(c == 0), stop=(c == NCHUNK - 1))

            LSET = pool.tile([64, P * SPR], F32)
            nc.scalar.activation(LSET, SET, ACT.Ln)
            LSE = pool.tile([P, TPP], F32)
            lse4 = LSE.rearrange("p (c g) -> p c g", g=CHUNK_T)
            for b in range(SPR):
                lb = psd.tile([P, 64], F32, tag="ta")
                nc.tensor.transpose(lb, LSET[:, P * b:P * (b + 1)], ident[0:64, 0:64])
                nc.scalar.activation(
                    lse4[:, :, 2 * b:2 * b + 2],
                    lb.rearrange("p (c t) -> p c t", t=2), ACT.Copy)

            lmdiff = pool.tile([P, TPP], F32)
            nc.vector.tensor_tensor(out=lmdiff, in0=LSE, in1=XL, op=ALU.subtract)
            nc.vector.scalar_tensor_tensor(
                out=junk_t(), in0=lmdiff, scalar=0.0, in1=mask01,
                op0=ALU.bypass, op1=ALU.mult,
                accum_out=stats[:, C_LMRW:C_LMRW + 1])

            # ---------- contract per-partition stats to per-row ----------
            RS = psum.tile([NROW, NSTAT], F32)
            nc.tensor.matmul(RS, rowsel, stats, start=True, stop=True)
            rs = pool.tile([NROW, NSTAT], F32)
            nc.scalar.activation(rs, RS, ACT.Copy)

            # ---------- per-row finals ----------
            q_b = smalls[:, 0:1]
            halted_b = smalls[:, 1:2]
            steps_b = smalls[:, 2:3]
            loss_cnt = rs[:, C_MASK:C_MASK + 1]
            cntcorr = rs[:, C_CORR:C_CORR + 1]
            lmraw = rs[:, C_LMRW:C_LMRW + 1]

            dvr = pool.tile([NROW, 1], F32)
            nc.vector.tensor_scalar(out=dvr, in0=loss_cnt, scalar1=1.0, scalar2=None,
                                    op0=ALU.max)
            rd = pool.tile([NROW, 1], F32)
            nc.vector.reciprocal(rd, dvr)

            valid = pool.tile([NROW, 1], F32)
            nc.vector.tensor_scalar(out=valid, in0=loss_cnt, scalar1=0.5, scalar2=None,
                                    op0=ALU.is_ge)
            nc.vector.tensor_tensor(out=valid, in0=valid, in1=halted_b, op=ALU.mult)

            # seq_is_correct: |cntcorr - loss_cnt| < 0.5
            sd = pool.tile([NROW, 1], F32)
            nc.vector.tensor_tensor(out=sd, in0=cntcorr, in1=loss_cnt, op=ALU.subtract)
            sg = pool.tile([NROW, 1], F32)
            nc.vector.tensor_scalar(out=sg, in0=sd, scalar1=-0.5, scalar2=None,
                                    op0=ALU.is_ge)
            sl_ = pool.tile([NROW, 1], F32)
            nc.vector.tensor_scalar(out=sl_, in0=sd, scalar1=0.5, scalar2=None,
                                    op0=ALU.is_le)
            seqc = pool.tile([NROW, 1], F32)
            nc.vector.tensor_tensor(out=seqc, in0=sg, in1=sl_, op=ALU.mult)

            # set_ok from lcnt/fcnt/pcnt
            lpos = pool.tile([NROW, 26], F32)
            nc.vector.tensor_scalar(out=lpos, in0=rs[:, C_LCNT:C_LCNT + 26],
                                    scalar1=0.5, scalar2=None, op0=ALU.is_ge)
            lneg = pool.tile([NROW, 26], F32)
            nc.vector.tensor_scalar(out=lneg, in0=rs[:, C_LCNT:C_LCNT + 26],
                                    scalar1=0.5, scalar2=None, op0=ALU.is_lt)
            psum26 = pool.tile([NROW, 26], F32)
            nc.vector.tensor_tensor(out=psum26, in0=rs[:, C_PCNT:C_PCNT + 26],
                                    in1=rs[:, C_PCNT2:C_PCNT2 + 26], op=ALU.add)
            ppos = pool.tile([NROW, 26], F32)
            nc.vector.tensor_scalar(out=ppos, in0=psum26,
                                    scalar1=0.5, scalar2=None, op0=ALU.is_ge)
            pneg = pool.tile([NROW, 26], F32)
            nc.vector.tensor_scalar(out=pneg, in0=psum26,
                                    scalar1=0.5, scalar2=None, op0=ALU.is_lt)
            fneg = pool.tile([NROW, 26], F32)
            nc.vector.tensor_scalar(out=fneg, in0=rs[:, C_FCNT:C_FCNT + 26],
                                    scalar1=0.5, scalar2=None, op0=ALU.is_lt)
            mis = pool.tile([NROW, 26], F32)
            nc.vector.tensor_tensor(out=mis, in0=lpos, in1=pneg, op=ALU.mult)
            hal = pool.tile([NROW, 26], F32)
            nc.vector.tensor_tensor(out=hal, in0=ppos, in1=lneg, op=ALU.mult)
            nc.vector.tensor_tensor(out=hal, in0=hal, in1=fneg, op=ALU.mult)
            nc.vector.tensor_tensor(out=mis, in0=mis, in1=hal, op=ALU.add)
            bad = pool.tile([NROW, 1], F32)
            nc.vector.reduce_max(bad, mis, axis=AX.X)
            setok = pool.tile([NROW, 1], F32)
            nc.vector.tensor_scalar(out=setok, in0=bad, scalar1=0.5, scalar2=None,
                                    op0=ALU.is_lt)

            # q-halt loss: softplus(q) - q*y ; y = seqc
            eq_ = pool.tile([NROW, 1], F32)
            nc.scalar.activation(eq_, q_b, ACT.Exp)
            nc.vector.tensor_scalar(out=eq_, in0=eq_, scalar1=1.0, scalar2=None,
                                    op0=ALU.add)
            sp = pool.tile([NROW, 1], F32)
            nc.scalar.activation(sp, eq_, ACT.Ln)
            qy = pool.tile([NROW, 1], F32)
            nc.vector.tensor_tensor(out=qy, in0=q_b, in1=seqc, op=ALU.mult)
            qh = pool.tile([NROW, 1], F32)
            nc.vector.tensor_tensor(out=qh, in0=sp, in1=qy, op=ALU.subtract)

            # qhalt metric: (q>=0) == seqc
            qge = pool.tile([NROW, 1], F32)
            nc.vector.tensor_scalar(out=qge, in0=q_b, scalar1=0.0, scalar2=None,
                                    op0=ALU.is_ge)
            qeq = pool.tile([NROW, 1], F32)
            nc.vector.tensor_tensor(out=qeq, in0=qge, in1=seqc, op=ALU.is_equal)

            out_t = pool.tile([NROW, NOUT], F32)
            nc.vector.memset(out_t, 0.0)
            nc.vector.tensor_tensor(out=out_t[:, 0:1], in0=lmraw, in1=rd, op=ALU.mult)
            nc.vector.tensor_copy(out_t[:, 1:2], qh)
            nc.vector.tensor_copy(out_t[:, 2:3], valid)
            acc = pool.tile([NROW, 1], F32)
            nc.vector.tensor_tensor(out=acc, in0=cntcorr, in1=rd, op=ALU.mult)
            nc.vector.tensor_tensor(out=acc, in0=acc, in1=valid, op=ALU.mult)
            nc.vector.tensor_copy(out_t[:, 3:4], acc)
            nc.vector.tensor_tensor(out=out_t[:, 4:5], in0=valid, in1=seqc, op=ALU.mult)
            nc.vector.tensor_tensor(out=out_t[:, 5:6], in0=valid, in1=setok, op=ALU.mult)
            nc.vector.tensor_tensor(out=out_t[:, 6:7], in0=valid, in1=qeq, op=ALU.mult)
            nc.vector.tensor_tensor(out=out_t[:, 7:8], in0=valid, in1=steps_b, op=ALU.mult)
            nc.sync.dma_start(out=out_d.ap(), in_=out_t)
    nc.compile()
    return nc


_NC_CACHE = None


def _get_nc():
    global _NC_CACHE
    if _NC_CACHE is None:
        _NC_CACHE = build_nc()
    return _NC_CACHE


def make_in_maps(logits, labels, q_halt_logits, halted, steps):
    in_maps = []
    for c in range(8):
        rs = slice(NROW * c, NROW * (c + 1))
        lg = np.ascontiguousarray(
            logits[rs].reshape(P, TPP * V).astype(np.float32, copy=False))
        lb = np.ascontiguousarray(
            labels[rs].reshape(P, TPP).astype(np.int32, copy=False))
        sm = np.zeros((NROW, 4), np.float32)
        sm[:, 0] = q_halt_logits[rs]
        sm[:, 1] = halted[rs].astype(np.float32)
        sm[:, 2] = steps[rs].astype(np.float32)
        in_maps.append({"logits": lg, "labels": lb, "smalls": sm})
    return in_maps


def combine(outs):
    """outs: list of 8 [16, NOUT] arrays -> reference-shaped 9-tuple."""
    a = np.concatenate(outs, axis=0).astype(np.float64)  # [128, NOUT]
    lm = a[:, 0].sum()
    qh = a[:, 1].sum()
    total = lm + 0.5 * qh
    m_count = int(round(a[:, 2].sum()))
    m_acc = a[:, 3].sum()
    m_exact = int(round(a[:, 4].sum()))
    m_set = int(round(a[:, 5].sum()))
    m_qhalt = int(round(a[:, 6].sum()))
    m_steps = int(round(a[:, 7].sum()))
    return (np.float32(total), np.float32(lm), np.float32(qh),
            np.int32(m_count), np.float32(m_acc), np.int32(m_exact),
            np.int32(m_set), np.int32(m_qhalt), np.int32(m_steps))


def kernel(logits, labels, q_halt_logits, halted, steps, trace=False):
    logits = np.asarray(logits)
    labels = np.asarray(labels)
    q_halt_logits = np.asarray(q_halt_logits)
    halted = np.asarray(halted)
    steps = np.asarray(steps)
    nc = _get_nc()
    in_maps = make_in_maps(logits, labels, q_halt_logits, halted, steps)
    res = run_bass_kernel_spmd(nc, in_maps, core_ids=list(range(8)), trace=trace)
    outs = [res.results[i]["out"] for i in range(8)]
    ret = combine(outs)
    if trace:
        return ret, res
    return ret
